# revision 14
# baseline (speedup 1.0000x reference)
"""CrossMerge kernel for trn2 — v8.

Math (per batch element):
    means_i = mean over C of g_i              (4, H, W)
    logits  = w_proj @ means + b_proj         (4, H, W)
    w       = softmax(logits, axis=1)         (4, H, W)
    out     = sum_i g_i * w_i                 (C, H, W)

Sharding: data-parallel over batch B=8 across 8 cores; weights replicated.

Staging: the host-side shard step packs the grids to bf16 (tolerance is
rel_err < 2e-2; bf16 internals land ~4e-3), so the device streams 18.9MB
of input + 9.4MB of fp32 output per core. Loads ride the sync/HWDGE
queue as plain u16 copies, two 1.05MB sub-DMAs per 1024-col block,
emitted two blocks ahead; stores ride the same queue behind them.

Engine budget per 512-col slice (~5.7us pace):
  PE   : 8 logits MMs (bf16 -> fp32 PSUM), 1 denom, 4 weight-broadcast
  ACT  : exp(L)+bias, 4 PSUM->SBUF bf16 weight copies
  DVE  : reciprocal, 5 products, 3 adds, 2 fp32 finals  (bf16 2x mode:
         plain 2-dim [128,512] APs only -- 3-dim/0-stride drop to 1x)
  Pool : W=E*R, grid-3 products, q2/s23 of chunk 1  (the Pool queue has
         NO DMAs on it -- SWDGE descriptor-generation head-blocks loads
         behind compute, measured as a 20us+ ramp stall)
  emission is software-pipelined (logits of slice s, denom of s-1,
  weights of s-2, products of s-3) so no in-order queue head-blocks.
"""

import os
import sys
from contextlib import ExitStack

import numpy as np

try:
    import concourse.bass as bass
except ImportError:  # fresh grading dir: concourse lives in the container repo
    sys.path.insert(0, "/opt/trn_rl_repo")
    import concourse.bass as bass

import concourse.tile as tile
from concourse import bacc, mybir
from concourse.bass_utils import run_bass_kernel_spmd

B, C, H, W = 8, 256, 96, 96
HW = H * W  # 9216
NCORES = 8
CPB = C // 128  # 2 partition chunks per core
JCOLS = 512  # softmax slice width (= fp32 PSUM bank)
NSL = HW // JCOLS  # 18 slices
BCOLS = 1024  # load/store block width
NBLK = HW // BCOLS  # 9

F32 = mybir.dt.float32
BF16 = mybir.dt.bfloat16
U16 = mybir.dt.uint16
AF = mybir.ActivationFunctionType

_CACHE = {}


def build_program():
    nc = bacc.Bacc("TRN2", debug=False, num_devices=NCORES)

    gall_d = nc.dram_tensor("gall", [4, C, HW], U16, kind="ExternalInput").ap()
    # bf16 constants, one blob: 0-15 ws | 16-19 ones4x4 | 20-531 selmat
    cbu_d = nc.dram_tensor("cbu", [128, 532], U16, kind="ExternalInput").ap()
    # fp32 constants: col 0 = exp bias (rows 0-3)
    cf_d = nc.dram_tensor("cf", [128, 1], F32, kind="ExternalInput").ap()
    out = nc.dram_tensor("out", [C, HW], F32, kind="ExternalOutput").ap()

    with tile.TileContext(nc) as tc, ExitStack() as ctx:
        const = ctx.enter_context(tc.tile_pool(name="const", bufs=1))
        gin = ctx.enter_context(tc.tile_pool(name="gin", bufs=5))
        outp = ctx.enter_context(tc.tile_pool(name="outp", bufs=3))
        narrow = ctx.enter_context(tc.tile_pool(name="narrow", bufs=4))
        wbsb = ctx.enter_context(tc.tile_pool(name="wbsb", bufs=3))
        prod = ctx.enter_context(tc.tile_pool(name="prod", bufs=3))
        ps_L = ctx.enter_context(tc.tile_pool(name="psL", bufs=2, space="PSUM"))
        ps_S4 = ctx.enter_context(tc.tile_pool(name="psS4", bufs=2, space="PSUM"))
        ps_Wb = ctx.enter_context(tc.tile_pool(name="psWb", bufs=1, space="PSUM"))

        cbu = const.tile([128, 532], U16)
        nc.sync.dma_start(out=cbu[:], in_=cbu_d)
        cb = cbu.bitcast(BF16)
        ws = cb[:, 0:16]
        ones4x4 = cb[0:4, 16:20]
        selmat = cb[0:4, 20:532]
        cf = const.tile([128, 1], F32)
        nc.sync.dma_start(out=cf[:], in_=cf_d)
        bv = cf[0:4, 0:1]

        gats = {}  # block -> gat tile
        st = {}  # slice -> dict of live tiles

        def emit_load(b):
            if not (0 <= b < NBLK):
                return
            gat = gin.tile([128, 4, CPB, BCOLS], BF16, tag="gall")
            for j in range(2):
                n0 = b * BCOLS + j * JCOLS
                nc.sync.dma_start(
                    out=gat[:, :, :, j * JCOLS : (j + 1) * JCOLS].bitcast(U16),
                    in_=gall_d[:, :, n0 : n0 + JCOLS].rearrange(
                        "i (c p) n -> p i c n", c=CPB
                    ),
                )
            gats[b] = gat

        # GpSimd first-op cold-start measured at 5-9us; absorb it up front.
        gwarm = narrow.tile([4, 16], BF16, tag="gwarm", bufs=1)
        nc.gpsimd.tensor_mul(gwarm[:], ws[0:4, :], ws[0:4, :])

        def xsl(s):
            return slice((s % 2) * JCOLS, (s % 2 + 1) * JCOLS)

        def p1_logits(s):
            if not (0 <= s < NSL):
                return
            gat = gats[s // 2]
            L = ps_L.tile([4, JCOLS], F32, tag="L")
            k = 0
            for i in range(4):
                for c in range(CPB):
                    nc.tensor.matmul(
                        L,
                        lhsT=ws[:, 4 * i : 4 * i + 4],
                        rhs=gat[:, i, c, xsl(s)],
                        start=(k == 0),
                        stop=(k == 7),
                    )
                    k += 1
            st[s] = {"L": L}

        def a1_exp(s):
            if not (0 <= s < NSL):
                return
            E = narrow.tile([4, JCOLS], BF16, tag="E")
            nc.scalar.activation(E[:], st[s]["L"], AF.Exp, bias=bv, scale=1.0)
            st[s]["E"] = E

        def p2_denom(s):
            if not (0 <= s < NSL):
                return
            S4 = ps_S4.tile([4, JCOLS], F32, tag="S4")
            nc.tensor.matmul(
                S4[:], lhsT=ones4x4, rhs=st[s]["E"][:], start=True, stop=True
            )
            st[s]["S4"] = S4

        def v_weights(s):
            if not (0 <= s < NSL):
                return
            # reciprocal DVE op requires base partition 0 (HW-verified in v1)
            R4 = narrow.tile([4, JCOLS], F32, tag="R4")
            nc.vector.reciprocal_approx_fast(R4[:], st[s]["S4"][:])
            W4 = narrow.tile([4, JCOLS], BF16, tag="W4")
            nc.gpsimd.tensor_mul(W4[:], st[s]["E"][:], R4[:])
            st[s]["W4"] = W4

        def p3_bcast(s):
            if not (0 <= s < NSL):
                return
            wbps = []
            for i in range(4):
                Wbp = ps_Wb.tile([128, JCOLS], F32, tag=f"wb{i}")
                nc.tensor.matmul(
                    Wbp[:],
                    lhsT=selmat[:, 128 * i : 128 * (i + 1)],
                    rhs=st[s]["W4"][:],
                    start=True,
                    stop=True,
                )
                wbps.append(Wbp)
            st[s]["wbp"] = wbps

        def a2_copies(s):
            if not (0 <= s < NSL):
                return
            wbs = []
            for i in range(4):
                Wb = wbsb.tile([128, JCOLS], BF16, tag=f"ws{i}", name=f"wbs{i}")
                nc.scalar.copy(Wb[:], st[s]["wbp"][i][:])
                wbs.append(Wb)
            st[s]["wbs"] = wbs

        ots = {}

        def products(s):
            if not (0 <= s < NSL):
                return
            b = s // 2
            gat = gats[b]
            wbs = st[s]["wbs"]
            if s % 2 == 0:
                ots[b] = outp.tile([128, CPB, BCOLS], F32, tag="ot", name="ot")
            ot = ots[b]
            for c in range(CPB):
                # chunk 1 runs its q2/q3/s23 on GpSimd to offload DVE; the
                # Pool queue carries no DMAs so this never stalls loads.
                e_q2 = nc.vector if c == 0 else nc.gpsimd
                q0 = prod.tile([128, JCOLS], BF16, tag="q0")
                nc.vector.tensor_mul(q0[:], gat[:, 0, c, xsl(s)], wbs[0][:])
                q1 = prod.tile([128, JCOLS], BF16, tag="q1")
                nc.vector.tensor_mul(q1[:], gat[:, 1, c, xsl(s)], wbs[1][:])
                s01 = prod.tile([128, JCOLS], BF16, tag="s01")
                nc.vector.tensor_add(s01[:], q0[:], q1[:])
                q2 = prod.tile([128, JCOLS], BF16, tag="q2")
                e_q2.tensor_mul(q2[:], gat[:, 2, c, xsl(s)], wbs[2][:])
                q3 = prod.tile([128, JCOLS], BF16, tag="q3")
                nc.gpsimd.tensor_mul(q3[:], gat[:, 3, c, xsl(s)], wbs[3][:])
                s23 = prod.tile([128, JCOLS], BF16, tag="s23")
                e_q2.tensor_add(s23[:], q2[:], q3[:])
                nc.vector.tensor_add(ot[:, c, xsl(s)], s01[:], s23[:])
            if s % 2 == 1:
                gats.pop(b)
                ot = ots.pop(b)
                N0 = b * BCOLS
                nc.sync.dma_start(
                    out=out[:, N0 : N0 + BCOLS].rearrange("(c p) n -> p c n", c=CPB),
                    in_=ot[:],
                )

        emit_load(0)
        emit_load(1)
        for s in range(NSL + 4):
            if s % 2 == 0:
                emit_load(s // 2 + 2)
            p1_logits(s)
            a1_exp(s - 1)
            p2_denom(s - 1)
            v_weights(s - 2)
            p3_bcast(s - 2)
            a2_copies(s - 2)
            products(s - 3)
            st.pop(s - 4, None)

    nc.compile()
    return nc


def _get_program():
    if "nc" not in _CACHE:
        _CACHE["nc"] = build_program()
    return _CACHE["nc"]


def _to_bf16_bits(x):
    """Round-to-nearest-even fp32 -> bf16 bit pattern (uint16)."""
    u = np.ascontiguousarray(x, dtype=np.float32).view(np.uint32)
    rounded = u + 0x7FFF + ((u >> 16) & 1)
    return (rounded >> 16).astype(np.uint16)


def make_consts(w_proj, b_proj):
    w = np.asarray(w_proj, dtype=np.float32)
    b = np.asarray(b_proj, dtype=np.float32)
    ws = np.empty((128, 16), dtype=np.float32)
    for i in range(4):
        for o in range(4):
            ws[:, 4 * i + o] = w[o, i] / C
    cbu = np.zeros((128, 532), dtype=np.float32)
    cbu[:, 0:16] = ws
    cbu[0:4, 16:20] = 1.0
    cbu[0:4, 20:532] = np.repeat(np.eye(4, dtype=np.float32), 128, axis=1)
    cf = np.zeros((128, 1), dtype=np.float32)
    cf[0:4, 0] = b
    return _to_bf16_bits(cbu), cf


LAST_RESULT = None


def kernel(g0, g1, g2, g3, w_proj, b_proj):
    global LAST_RESULT
    nc = _get_program()

    cbu, cf = make_consts(w_proj, b_proj)

    gall = np.stack(
        [np.asarray(x, dtype=np.float32).reshape(B, C, HW) for x in (g0, g1, g2, g3)],
        axis=1,
    )  # (B, 4, C, HW)
    gbits = _to_bf16_bits(gall)
    in_maps = []
    for bi in range(NCORES):
        m = {"gall": np.ascontiguousarray(gbits[bi]), "cbu": cbu, "cf": cf}
        in_maps.append(m)

    res = run_bass_kernel_spmd(
        nc,
        in_maps,
        list(range(NCORES)),
        trace=bool(int(os.environ.get("CM_TRACE", "0"))),
        tmpdir=os.environ.get("CM_TRACE_DIR") or None,
    )
    LAST_RESULT = res
    out_full = np.stack(
        [res.results[bi]["out"].reshape(C, H, W) for bi in range(NCORES)], axis=0
    )
    return out_full
